# revision 42
# baseline (speedup 1.0000x reference)
"""Trainium2 Bass kernel for nn_Attention (B=4, T=2048, D=1024, H=16, dk=dv=64).

Sharding: 8 cores = 4 batch shards x 2 head-groups (tensor parallel).
Each core computes, for its (batch b, head-group g):
    partial_out[b,g] = attention(x_b, W*_g)  @ Wo_g      (no bo)
Host sums the two head-group partials per batch and adds bo.

All shapes/sharding are hardcoded (self-contained; no sibling imports).
Compute dtype: bf16 matmuls with fp32 PSUM accumulation; bf16 output
partials (summed in fp32 on host).

v3 schedule:
- host packs [x^T(tok 0:1024) | Wq | Wk | Wv] into one per-kt strip so
  the input stream needs ~30 DMA dispatches instead of ~60 (dispatch is
  ~0.65us each on the sync queue and gates startup).
- kt-outer boot phase accumulates Q/K(ct0,tq0-1) + V(tt0-3) across all
  8 PSUM banks while strips stream in; PE busy from first strip.
- filler units (remaining projections) carry (stage, wave): stage =
  last (pr,qb) index before which they must run, wave = which input
  wave their data arrives in; drip pops readiness-ordered, flush scans
  stage-ordered.
- softmax normalize: ctx/l leave PSUM via DVE (even) + ACT (odd) copies
  in parallel, fast approximate reciprocal, scale mults deferred behind
  the next chunk's copies so the DVE FIFO never delays PSUM recycling.
- out-projection: both 512-halves accumulate in one PSUM tile, copies
  on DVE+GpSimd (keeps ACT free for exp), single 1024-wide DMA per row
  block.
"""

import numpy as np

# Problem constants (hardcoded per contract)
B, T, D_MODEL, NUM_HEADS, D_K = 4, 2048, 1024, 16, 64
HG = 2                      # head groups (tensor parallel)
CH = NUM_HEADS * D_K // HG  # 512 channels per group (8 heads)
H_LOC = NUM_HEADS // HG     # 8 heads per core
N_CORES = 8
P = 128                     # partitions
KT = D_MODEL // P           # 8 k-tiles over d_model
CT = CH // P                # 4 channel tiles (= head PAIRS)
TT = T // P                 # 16 token tiles
TQB = 512                   # query-chunk (matmul free dim)
NQ = T // TQB               # 4 query chunks
SCALE = 1.0 / 8.0           # 1/sqrt(dk)
TH = T // 2                 # strip covers tokens [0:TH)
SW = TH + 3 * CH            # strip width: 1024 + 3*512 = 2560

_NC_CACHE = None


def build_program():
    import concourse.bass as bass
    import concourse.mybir as mybir
    import concourse.tile as tile
    from concourse import bacc

    fp32 = mybir.dt.float32
    bf16 = mybir.dt.bfloat16
    AF = mybir.ActivationFunctionType
    ALU = mybir.AluOpType

    nc = bacc.Bacc(
        "TRN2",
        target_bir_lowering=False,
        debug=False,
        enable_asserts=False,
        num_devices=N_CORES,
    )

    # DRAM I/O (per-core shards)
    strip = nc.dram_tensor("strip", [D_MODEL, SW], bf16, kind="ExternalInput").ap()
    xT2 = nc.dram_tensor("xT2", [D_MODEL, T - TH], bf16, kind="ExternalInput").ap()
    wo = nc.dram_tensor("wo", [CH, D_MODEL], bf16, kind="ExternalInput").ap()
    bq_p = nc.dram_tensor("bq_p", [P, CT], fp32, kind="ExternalInput").ap()
    bk_p = nc.dram_tensor("bk_p", [P, CT], fp32, kind="ExternalInput").ap()
    msk = nc.dram_tensor("msk", [P, P], bf16, kind="ExternalInput").ap()
    perm = nc.dram_tensor("perm", [P, P], fp32, kind="ExternalInput").ap()
    outp = nc.dram_tensor("outp", [T, D_MODEL], bf16, kind="ExternalOutput").ap()

    strip_r = strip.rearrange("(kt p) c -> kt p c", p=P)
    xT2_r = xT2.rearrange("(kt p) c -> kt p c", p=P)
    wo_r = wo.rearrange("(ct p) c -> ct p c", p=P)
    outp_r = outp.rearrange("(tt p) c -> tt p c", p=P)

    with tile.TileContext(nc) as tc, \
         tc.tile_pool(name="persist", bufs=1) as pp:
        # Persistent SBUF tensors (one slot each; tag defaults to name)
        st_sb = pp.tile([P, KT, SW], bf16, name="st_sb")
        xT2_sb = pp.tile([P, KT, T - TH], bf16, name="xT2_sb")
        wo_sb = pp.tile([P, CT, D_MODEL], bf16, name="wo_sb")
        bq_sb = pp.tile([P, CT], fp32, name="bq_sb")
        bk_sb = pp.tile([P, CT], fp32, name="bk_sb")
        msk_sb = pp.tile([P, P], bf16, name="msk_sb")
        perm_sb = pp.tile([P, P], fp32, name="perm_sb")
        QT_sb = pp.tile([P, CT, T], bf16, name="QT_sb")   # head h -> parts 64*(h%2)+, idx h//2
        KT_sb = pp.tile([P, CT, T], bf16, name="KT_sb")
        # V' with interleaved ones columns:
        #  even h: cols [0:64]=V_h,  [64:128]=1  -> ctx rows 0:64,  l rows 64:128
        #  odd  h: cols [0:64]=1, [64:128]=V_h   -> l rows 0:64,  ctx rows 64:128
        Vp_sb = pp.tile([P, TT, H_LOC, P], bf16, name="Vp_sb")
        cxT_sb = pp.tile([P, CT, T], bf16, name="cxT_sb")

        # views into the packed strip
        def xv(kt, lo, hi):
            """x^T[:, kt, token lo:hi] (lo/hi multiples of 512)."""
            if hi <= TH:
                return st_sb[:, kt, lo:hi]
            assert lo >= TH
            return xT2_sb[:, kt, lo - TH:hi - TH]

        def wqv(kt, lo, hi):
            return st_sb[:, kt, TH + lo:TH + hi]

        def wkv(kt, lo, hi):
            return st_sb[:, kt, TH + CH + lo:TH + CH + hi]

        def wvv(kt):
            return st_sb[:, kt, TH + 2 * CH:TH + 3 * CH]

        # --- input DMAs: few dispatches, boot-ordered ----------------------
        for kt in range(2):
            nc.sync.dma_start(st_sb[:, kt], strip_r[kt])
        nc.sync.dma_start(msk_sb[:], msk)
        nc.sync.dma_start(bq_sb[:], bq_p)
        nc.sync.dma_start(bk_sb[:], bk_p)
        for kt in range(2, KT):
            nc.sync.dma_start(st_sb[:, kt], strip_r[kt])
        TQ = T // 4
        for kt in range(KT):  # tokens 1024:1536
            nc.sync.dma_start(xT2_sb[:, kt, 0:TQ], xT2_r[kt, :, 0:TQ])
        for kt in range(KT):  # tokens 1536:2048
            nc.sync.dma_start(xT2_sb[:, kt, TQ:2 * TQ], xT2_r[kt, :, TQ:2 * TQ])
        for ct in range(CT):
            nc.sync.dma_start(wo_sb[:, ct], wo_r[ct])
        nc.sync.dma_start(perm_sb[:], perm)

        # ones columns of V'
        for h in range(H_LOC):
            off = 64 if h % 2 == 0 else 0
            nc.gpsimd.memset(Vp_sb[:, :, h, off:off + 64], 1.0)

        # --- boot phase: kt-outer projection fills all 8 PSUM banks --------
        # Q(ct0, tq0-1), K(ct0, tq0-1): 2 banks each; V(tt0-3): 4 banks.
        with tc.tile_pool(name="boot", bufs=1, space="PSUM") as bp:
            qb_ps = bp.tile([P, 2 * TQB], fp32, name="qb_ps")
            kb_ps = bp.tile([P, 2 * TQB], fp32, name="kb_ps")
            vb_ps = bp.tile([P, 4, CH], fp32, name="vb_ps")
            for kt in range(KT):
                st, sp = (kt == 0), (kt == KT - 1)
                for tq in range(2):
                    ts = slice(tq * TQB, (tq + 1) * TQB)
                    nc.tensor.matmul(
                        qb_ps[:, ts], lhsT=wqv(kt, 0, P),
                        rhs=xv(kt, tq * TQB, (tq + 1) * TQB), start=st, stop=sp,
                        skip_group_check=True)
                    nc.tensor.matmul(
                        kb_ps[:, ts], lhsT=wkv(kt, 0, P),
                        rhs=xv(kt, tq * TQB, (tq + 1) * TQB), start=st, stop=sp,
                        skip_group_check=True)
                for tt in range(4):
                    nc.tensor.matmul(
                        vb_ps[:, tt], lhsT=xv(kt, tt * P, (tt + 1) * P),
                        rhs=wvv(kt), start=st, stop=sp,
                        skip_group_check=True)
            # finalize: Q/K biases on DVE, V copies split ACT/DVE (parallel)
            # so the PSUM banks recycle into the attention pools fast. V
            # carries no bias here: bv@Wo is folded into bo on the host.
            VpR = Vp_sb.rearrange("p tt (hp two) c -> p tt hp two c", two=2)
            v_r = vb_ps.rearrange("p tt (hp two c) -> p tt hp two c",
                                  two=2, c=64)
            nc.scalar.copy(out=VpR[:, 0:4, :, 0, 0:64], in_=v_r[:, :, :, 0, :])
            nc.scalar.copy(out=VpR[:, 0:4, :, 1, 64:128], in_=v_r[:, :, :, 1, :])
            for tq in range(2):
                ts = slice(tq * TQB, (tq + 1) * TQB)
                nc.vector.tensor_scalar_add(QT_sb[:, 0, ts], qb_ps[:, ts],
                                            bq_sb[:, 0:1])
            for tq in range(2):
                ts = slice(tq * TQB, (tq + 1) * TQB)
                nc.vector.tensor_scalar_add(KT_sb[:, 0, ts], kb_ps[:, ts],
                                            bk_sb[:, 0:1])

        with tc.tile_pool(name="psA", bufs=2, space="PSUM") as psA, \
             tc.tile_pool(name="psB", bufs=1, space="PSUM") as psB, \
             tc.tile_pool(name="wp", bufs=3) as wp:

            # ---- emission units (generators: yield every ~2 matmuls so the
            # drip scheduler can interleave filler at per-kb granularity) ---
            def emit_qk_unit(which, ct, tq):
                """One [128ch, 512tok] projection tile of Q^T or K^T."""
                lo, hi = tq * TQB, (tq + 1) * TQB
                wv_, b_sb, dst = ((wqv, bq_sb, QT_sb) if which == "q"
                                  else (wkv, bk_sb, KT_sb))
                p_ps = psA.tile([P, 2 * TQB], fp32, tag="s2", bufs=3, name="p_ps")[:, :TQB]
                for kt in range(KT):
                    nc.tensor.matmul(
                        p_ps, lhsT=wv_(kt, ct * P, (ct + 1) * P),
                        rhs=xv(kt, lo, hi),
                        start=(kt == 0), stop=(kt == KT - 1))
                    if kt % 2 == 1 and kt < KT - 1:
                        yield
                nc.vector.tensor_scalar_add(dst[:, ct, lo:hi], p_ps, b_sb[:, ct:ct + 1])

            def emit_v_unit(tt):
                """One [128tok, 512ch] V tile scattered into Vp (bias-free)."""
                v_ps = psA.tile([P, 2 * TQB], fp32, tag="s2", bufs=3, name="v_ps")[:, :CH]
                for kt in range(KT):
                    nc.tensor.matmul(
                        v_ps, lhsT=xv(kt, tt * P, (tt + 1) * P),
                        rhs=wvv(kt),
                        start=(kt == 0), stop=(kt == KT - 1))
                    if kt % 2 == 1 and kt < KT - 1:
                        yield
                v_r = v_ps.rearrange("p (hp two c) -> p hp two c", two=2, c=64)
                VpR2 = Vp_sb.rearrange("p tt (hp two) c -> p tt hp two c", two=2)
                nc.vector.tensor_copy(out=VpR2[:, tt, :, 0, 0:64], in_=v_r[:, :, 0, :])
                nc.vector.tensor_copy(out=VpR2[:, tt, :, 1, 64:128], in_=v_r[:, :, 1, :])

            def emit_outproj_unit(tt):
                """Both 512-halves of one [128tok, 1024] output row block.
                ct CT-1 accumulates last: its cxT may come from the chunk
                that just finished, whose scale-mults are still in flight."""
                o_ps = psA.tile([P, 2 * TQB], fp32, tag="s2", bufs=3, name="o_ps")
                for ct in range(CT):
                    for nh in range(2):
                        hs = slice(nh * TQB, (nh + 1) * TQB)
                        nc.tensor.matmul(
                            o_ps[:, hs], lhsT=cxT_sb[:, ct, tt * P:(tt + 1) * P],
                            rhs=wo_sb[:, ct, hs],
                            start=(ct == 0), stop=(ct == CT - 1),
                            skip_group_check=True)
                    if ct < CT - 1:
                        yield
                ob = wp.tile([P, 2 * TQB], bf16, tag="ob", bufs=3, name="ob")
                nc.vector.tensor_copy(out=ob, in_=o_ps)
                nc.sync.dma_start(outp_r[tt], ob)

            # filler queue: (stage, wave, genfn). stage = last chunk index
            # before which the unit must complete; wave = input wave the
            # unit's data arrives in (drip order is readiness-major).
            filler = []
            cur = [None]  # in-progress generator

            def _finish(gen):
                for _ in gen:
                    pass

            def drip(n=1):
                """Advance filler emission by n micro-steps (~2 matmuls)."""
                while n > 0:
                    if cur[0] is None:
                        if not filler:
                            return
                        _, _, mk = filler.pop(0)
                        cur[0] = mk()
                    try:
                        next(cur[0])
                        n -= 1
                    except StopIteration:
                        cur[0] = None

            def drip_boundary():
                """Chunk-boundary cover (~2us): finish the in-flight unit,
                then run one fresh unit. A second fresh allocation would just
                block on the s2 rotation until the next exp completes."""
                if cur[0] is not None:
                    _finish(cur[0])
                    cur[0] = None
                if filler:
                    _, _, mk = filler.pop(0)
                    _finish(mk())

            def flush(stage):
                if cur[0] is not None:
                    _finish(cur[0])
                    cur[0] = None
                i = 0
                while i < len(filler):
                    if filler[i][0] <= stage:
                        _, _, mk = filler.pop(i)
                        _finish(mk())
                    else:
                        i += 1

            # ---- attention ---------------------------------------------
            def emit_s_pair(pr, qb, kb):
                ks = slice(kb * P, (kb + 1) * P)
                v = kb - 4 * qb
                qoff = 128 * v if v > 0 else 0
                qsn = slice(qb * TQB + qoff, (qb + 1) * TQB)
                s2 = psA.tile([P, 2 * TQB], fp32, tag="s2", bufs=3, name="s2")
                nc.tensor.matmul(
                    s2[:, qoff:TQB],
                    lhsT=KT_sb[0:64, pr, ks], rhs=QT_sb[0:64, pr, qsn],
                    start=True, stop=True)
                nc.tensor.matmul(
                    s2[:, TQB + qoff:2 * TQB],
                    lhsT=KT_sb[64:128, pr, ks], rhs=QT_sb[64:128, pr, qsn],
                    start=True, stop=True)
                return s2

            def emit_exp_ctx(pr, qb, kb, nkb, s2, cx_e, cx_o):
                he, ho = 2 * pr, 2 * pr + 1
                v = kb - 4 * qb
                qoff = 128 * v if v > 0 else 0
                s2r = s2.rearrange("p (two c) -> p two c", two=2)
                e2 = wp.tile([P, 2, TQB], bf16, tag="e2", bufs=8, name="e2")
                nc.scalar.activation(
                    e2[:, :, qoff:TQB], s2r[:, :, qoff:TQB], AF.Exp, scale=SCALE)
                if v >= 0:  # diagonal: triangular mask on first 128 q-cols
                    nc.gpsimd.tensor_tensor(
                        e2[:, :, qoff:qoff + P], e2[:, :, qoff:qoff + P],
                        msk_sb[:, None, :].to_broadcast((P, 2, P)), ALU.mult)
                nc.tensor.matmul(
                    cx_e[:, qoff:TQB], lhsT=Vp_sb[:, kb, he, :],
                    rhs=e2[:, 0, qoff:TQB],
                    start=(kb == 0), stop=(kb == nkb - 1), skip_group_check=True)
                nc.tensor.matmul(
                    cx_o[:, qoff:TQB], lhsT=Vp_sb[:, kb, ho, :],
                    rhs=e2[:, 1, qoff:TQB],
                    start=(kb == 0), stop=(kb == nkb - 1), skip_group_check=True)

            deferred_mults = []

            def emit_attention(pr, qb, fine_tail=False):
                qs = slice(qb * TQB, (qb + 1) * TQB)
                nkb = 4 * qb + 4
                cx = psB.tile([P, 2, TQB], fp32, tag="cx", name="cx")
                cx_e, cx_o = cx[:, 0], cx[:, 1]
                pend = [emit_s_pair(pr, qb, kb) for kb in range(min(2, nkb))]
                # cover the previous normalize / first-exp latency bubble
                drip_boundary()
                for kb in range(nkb):
                    if kb + 2 < nkb:
                        pend.append(emit_s_pair(pr, qb, kb + 2))
                    emit_exp_ctx(pr, qb, kb, nkb, pend[kb], cx_e, cx_o)
                    if kb < nkb - 2 and kb % 2 == 0:
                        drip(1)
                # normalize: stage ctx/l out of PSUM in one wide DVE copy
                # (frees both cx banks; keeps ACT exp-only);
                # even head: ctx rows 0:64, l rows 64:128 / odd head mirrored.
                csb = wp.tile([P, 2, TQB], fp32, tag="csb", bufs=2, name="csb")
                nc.vector.tensor_copy(out=csb, in_=cx)
                cse, cso = csb[:, 0], csb[:, 1]
                lpair = wp.tile([P, TQB], fp32, tag="lpair", bufs=2, name="lpair")
                nc.vector.tensor_copy(out=lpair[64:128], in_=cse[64:128])
                nc.vector.tensor_copy(out=lpair[0:64], in_=cso[0:64])
                rec = wp.tile([P, TQB], fp32, tag="rec", bufs=2, name="rec")
                nc.vector.reciprocal_approx_fast(rec, lpair)
                if not fine_tail:
                    recs = wp.tile([P, TQB], fp32, tag="recs", bufs=2, name="recs")
                    nc.sync.dma_start(recs[0:64], rec[64:128])
                    nc.sync.dma_start(recs[64:128], rec[0:64])
                # previous qb's deferred scaling mults run AFTER this qb's
                # copies/recips so the DVE FIFO frees cx banks first
                while deferred_mults:
                    deferred_mults.pop(0)()

                if fine_tail:
                    # Last chunk (pr == CT-1): shorten the tail.
                    # - 1/l partition swap via a PE permutation matmul
                    #   instead of the slow SBUF-SBUF DMA
                    # - out-projection accumulates ct 0..2 early (their cxT
                    #   is old) and this chunk's ct last, per 128-row block
                    def op_early(tt):
                        o_ps = psA.tile([P, 2 * TQB], fp32, tag="s2",
                                        bufs=3, name="o_fin")
                        for nh in range(2):
                            hs = slice(nh * TQB, (nh + 1) * TQB)
                            for ct in range(CT - 1):
                                nc.tensor.matmul(
                                    o_ps[:, hs],
                                    lhsT=cxT_sb[:, ct, tt * P:(tt + 1) * P],
                                    rhs=wo_sb[:, ct, hs],
                                    start=(ct == 0), stop=False,
                                    skip_group_check=True)
                        return o_ps

                    def op_late(tt, o_ps):
                        for nh in range(2):
                            hs = slice(nh * TQB, (nh + 1) * TQB)
                            nc.tensor.matmul(
                                o_ps[:, hs],
                                lhsT=cxT_sb[:, CT - 1, tt * P:(tt + 1) * P],
                                rhs=wo_sb[:, CT - 1, hs],
                                start=False, stop=True,
                                skip_group_check=True)
                        ob = wp.tile([P, 2 * TQB], bf16, tag="ob", bufs=3,
                                     name="ob")
                        nc.vector.tensor_copy(out=ob, in_=o_ps)
                        nc.sync.dma_start(outp_r[tt], ob)

                    o01 = [op_early(4 * qb), op_early(4 * qb + 1)]
                    recs_ps = psB.tile([P, 2, TQB], fp32, tag="cx",
                                       name="recs_ps")[:, 0]
                    nc.tensor.matmul(recs_ps, lhsT=perm_sb, rhs=rec,
                                     start=True, stop=True)
                    for tf in range(4):
                        ts_ = slice(qb * TQB + tf * P, qb * TQB + (tf + 1) * P)
                        fs = slice(tf * P, (tf + 1) * P)
                        nc.vector.tensor_tensor(
                            cxT_sb[0:64, pr, ts_], cse[0:64, fs],
                            recs_ps[0:64, fs], ALU.mult)
                        nc.vector.tensor_tensor(
                            cxT_sb[64:128, pr, ts_], cso[64:128, fs],
                            recs_ps[64:128, fs], ALU.mult)
                        o_ps = o01[tf] if tf < 2 else op_early(4 * qb + tf)
                        op_late(4 * qb + tf, o_ps)
                    return

                def mults(pr=pr, qs=qs, cse=cse, cso=cso, recs=recs):
                    nc.vector.tensor_tensor(
                        cxT_sb[0:64, pr, qs], cse[0:64], recs[0:64], ALU.mult)
                    nc.vector.tensor_tensor(
                        cxT_sb[64:128, pr, qs], cso[64:128], recs[64:128], ALU.mult)
                deferred_mults.append(mults)

            # ---- schedule ----------------------------------------------
            # qb-major chunk order: all (pr, qb<=1) chunks first (these need
            # only boot + strip data), then qb=2, then qb=3. This keeps the
            # PE off the late xT waves early, and frees out-proj filler work
            # for the late phases.
            chunks = ([(pr, qb) for pr in range(CT) for qb in (0, 1)]
                      + [(pr, 2) for pr in range(CT)]
                      + [(pr, 3) for pr in range(CT)])
            pos = {c: i for i, c in enumerate(chunks)}

            # boot covered Q/K(ct0, tq0-1) and V(tt0-3).
            # wave: 0 = strip data (tok<1024), 1 = tok 1024:1536, 2 = rest.
            for tq in (2, 3):
                filler.append((pos[(0, tq)], tq - 1,
                               lambda tq=tq: emit_qk_unit("q", 0, tq)))
                filler.append((pos[(0, tq)], tq - 1,
                               lambda tq=tq: emit_qk_unit("k", 0, tq)))
            for tt in range(4, TT):
                wave = 0 if tt < 8 else (1 if tt < 12 else 2)
                filler.append((pos[(0, tt // 4)], wave,
                               lambda tt=tt: emit_v_unit(tt)))
            for ct in range(1, CT):
                for tq in range(NQ):
                    wave = 0 if tq < 2 else tq - 1
                    filler.append((pos[(ct, tq)], wave,
                                   lambda tq=tq, ct=ct: emit_qk_unit("q", ct, tq)))
                    filler.append((pos[(ct, tq)], wave,
                                   lambda tq=tq, ct=ct: emit_qk_unit("k", ct, tq)))
            filler.sort(key=lambda sf: (sf[1], sf[0]))

            for ci, (pr, qb) in enumerate(chunks):
                flush(ci)
                last = ci == len(chunks) - 1
                if last:
                    while filler:
                        drip()
                emit_attention(pr, qb, fine_tail=last)
                if pr == CT - 1 and qb < NQ - 1:
                    # rows for this qb are now complete across all heads;
                    # emit this chunk's deferred cxT mults before any out-proj
                    # filler can be dripped against them
                    while deferred_mults:
                        deferred_mults.pop(0)()
                    for tt in range(4 * qb, 4 * qb + 4):
                        filler.append((99, 9, lambda tt=tt: emit_outproj_unit(tt)))

    nc.compile()
    return nc


def _get_nc():
    global _NC_CACHE
    if _NC_CACHE is None:
        _NC_CACHE = build_program()
    return _NC_CACHE


def _shard_inputs(input_Q, mask, Wq, bq, Wk, bk, Wv, bv, Wo):
    import ml_dtypes
    bf16 = ml_dtypes.bfloat16
    f32 = np.float32

    input_Q = np.asarray(input_Q, dtype=f32)
    mask = np.asarray(mask, dtype=bool)
    # causal-structure check: masks are derived assuming block-Toeplitz causal mask
    assert np.array_equal(mask, np.triu(np.ones((T, T), dtype=bool), k=1)), \
        "kernel assumes the standard causal mask"

    # triangular mask tile [128, 128]: keep iff p <= f  (k-offset p, q-offset f)
    keep = (~mask[0:P, 0:P]).astype(f32)              # [q, k]
    msk_np = np.ascontiguousarray(keep.T.astype(bf16))

    in_maps = []
    for c in range(N_CORES):
        b, g = c // HG, c % HG
        cs = slice(g * CH, (g + 1) * CH)
        xT_np = input_Q[b].T.astype(bf16)             # [D, T]
        wq_np = np.asarray(Wq, f32)[:, cs].astype(bf16)
        wk_np = np.asarray(Wk, f32)[:, cs].astype(bf16)
        wv_np = np.asarray(Wv, f32)[:, cs].astype(bf16)
        strip_np = np.ascontiguousarray(
            np.concatenate([xT_np[:, 0:TH], wq_np, wk_np, wv_np], axis=1))
        in_maps.append({
            "strip": strip_np,
            "xT2": np.ascontiguousarray(xT_np[:, TH:T]),
            "wo": np.ascontiguousarray(np.asarray(Wo, f32)[cs, :].astype(bf16)),
            "bq_p": np.ascontiguousarray(np.asarray(bq, f32)[cs].reshape(CT, P).T),
            "bk_p": np.ascontiguousarray(np.asarray(bk, f32)[cs].reshape(CT, P).T),
            "msk": msk_np,
            "perm": np.ascontiguousarray(
                np.roll(np.eye(P, dtype=f32), P // 2, axis=0)),
        })
    return in_maps


def _run(inputs, trace=False):
    from concourse.bass_utils import run_bass_kernel_spmd

    nc = _get_nc()
    in_maps = _shard_inputs(
        inputs["input_Q"], inputs["mask"], inputs["Wq"], inputs["bq"],
        inputs["Wk"], inputs["bk"], inputs["Wv"], inputs["bv"], inputs["Wo"])
    res = run_bass_kernel_spmd(
        nc, in_maps, core_ids=list(range(N_CORES)), trace=trace)

    # bv is folded here: concat_h(ctx_h/l_h + bv_h) @ Wo
    #                  = concat_h(ctx_h/l_h) @ Wo + bv @ Wo
    bo = (np.asarray(inputs["bo"], np.float32)
          + np.asarray(inputs["bv"], np.float32)
          @ np.asarray(inputs["Wo"], np.float32))
    out = np.empty((B, T, D_MODEL), np.float32)
    for b in range(B):
        out[b] = (np.asarray(res.results[2 * b]["outp"], np.float32)
                  + np.asarray(res.results[2 * b + 1]["outp"], np.float32)
                  + bo)
    return out, res


def kernel(**inputs):
    out, _ = _run(inputs, trace=False)
    return out


# revision 44
# speedup vs baseline: 1.1900x; 1.1900x over previous
"""Trainium2 Bass kernel for nn_Attention (B=4, T=2048, D=1024, H=16, dk=dv=64).

Sharding: 8 cores = 4 batch shards x 2 head-groups (tensor parallel).
Each core computes, for its (batch b, head-group g):
    partial_out[b,g] = attention(x_b, W*_g)  @ Wo_g      (no bo)
Host sums the two head-group partials per batch and adds bo.

All shapes/sharding are hardcoded (self-contained; no sibling imports).
Compute dtype: bf16 matmuls with fp32 PSUM accumulation; bf16 output
partials (summed in fp32 on host).

v3 schedule:
- host packs [x^T(tok 0:1024) | Wq | Wk | Wv] into one per-kt strip so
  the input stream needs ~30 DMA dispatches instead of ~60 (dispatch is
  ~0.65us each on the sync queue and gates startup).
- kt-outer boot phase accumulates Q/K(ct0,tq0-1) + V(tt0-3) across all
  8 PSUM banks while strips stream in; PE busy from first strip.
- filler units (remaining projections) carry (stage, wave): stage =
  last (pr,qb) index before which they must run, wave = which input
  wave their data arrives in; drip pops readiness-ordered, flush scans
  stage-ordered.
- softmax normalize: ctx/l leave PSUM via DVE (even) + ACT (odd) copies
  in parallel, fast approximate reciprocal, scale mults deferred behind
  the next chunk's copies so the DVE FIFO never delays PSUM recycling.
- out-projection: both 512-halves accumulate in one PSUM tile, copies
  on DVE+GpSimd (keeps ACT free for exp), single 1024-wide DMA per row
  block.
"""

import numpy as np

# Problem constants (hardcoded per contract)
B, T, D_MODEL, NUM_HEADS, D_K = 4, 2048, 1024, 16, 64
HG = 2                      # head groups (tensor parallel)
CH = NUM_HEADS * D_K // HG  # 512 channels per group (8 heads)
H_LOC = NUM_HEADS // HG     # 8 heads per core
N_CORES = 8
P = 128                     # partitions
KT = D_MODEL // P           # 8 k-tiles over d_model
CT = CH // P                # 4 channel tiles (= head PAIRS)
TT = T // P                 # 16 token tiles
TQB = 512                   # query-chunk (matmul free dim)
NQ = T // TQB               # 4 query chunks
SCALE = 1.0 / 8.0           # 1/sqrt(dk)
TH = T // 2                 # strip covers tokens [0:TH)
SW = TH + 3 * CH            # strip width: 1024 + 3*512 = 2560

_NC_CACHE = None


def build_program():
    import concourse.bass as bass
    import concourse.mybir as mybir
    import concourse.tile as tile
    from concourse import bacc

    fp32 = mybir.dt.float32
    bf16 = mybir.dt.bfloat16
    AF = mybir.ActivationFunctionType
    ALU = mybir.AluOpType

    nc = bacc.Bacc(
        "TRN2",
        target_bir_lowering=False,
        debug=False,
        enable_asserts=False,
        num_devices=N_CORES,
    )

    # DRAM I/O (per-core shards)
    strip = nc.dram_tensor("strip", [D_MODEL, SW], bf16, kind="ExternalInput").ap()
    xT2 = nc.dram_tensor("xT2", [D_MODEL, T - TH], bf16, kind="ExternalInput").ap()
    wo = nc.dram_tensor("wo", [CH, D_MODEL], bf16, kind="ExternalInput").ap()
    bq_p = nc.dram_tensor("bq_p", [P, CT], fp32, kind="ExternalInput").ap()
    bk_p = nc.dram_tensor("bk_p", [P, CT], fp32, kind="ExternalInput").ap()
    msk = nc.dram_tensor("msk", [P, P], bf16, kind="ExternalInput").ap()
    perm = nc.dram_tensor("perm", [P, P], fp32, kind="ExternalInput").ap()
    outp = nc.dram_tensor("outp", [T, D_MODEL], bf16, kind="ExternalOutput").ap()

    strip_r = strip.rearrange("(kt p) c -> kt p c", p=P)
    xT2_r = xT2.rearrange("(kt p) c -> kt p c", p=P)
    wo_r = wo.rearrange("(ct p) c -> ct p c", p=P)
    outp_r = outp.rearrange("(tt p) c -> tt p c", p=P)

    with tile.TileContext(nc) as tc, \
         tc.tile_pool(name="persist", bufs=1) as pp:
        # Persistent SBUF tensors (one slot each; tag defaults to name)
        st_sb = pp.tile([P, KT, SW], bf16, name="st_sb")
        xT2_sb = pp.tile([P, KT, T - TH], bf16, name="xT2_sb")
        wo_sb = pp.tile([P, CT, D_MODEL], bf16, name="wo_sb")
        bq_sb = pp.tile([P, CT], fp32, name="bq_sb")
        bk_sb = pp.tile([P, CT], fp32, name="bk_sb")
        msk_sb = pp.tile([P, P], bf16, name="msk_sb")
        perm_sb = pp.tile([P, P], fp32, name="perm_sb")
        QT_sb = pp.tile([P, CT, T], bf16, name="QT_sb")   # head h -> parts 64*(h%2)+, idx h//2
        KT_sb = pp.tile([P, CT, T], bf16, name="KT_sb")
        # V' with interleaved ones columns:
        #  even h: cols [0:64]=V_h,  [64:128]=1  -> ctx rows 0:64,  l rows 64:128
        #  odd  h: cols [0:64]=1, [64:128]=V_h   -> l rows 0:64,  ctx rows 64:128
        Vp_sb = pp.tile([P, TT, H_LOC, P], bf16, name="Vp_sb")
        cxT_sb = pp.tile([P, CT, T], bf16, name="cxT_sb")

        # views into the packed strip
        def xv(kt, lo, hi):
            """x^T[:, kt, token lo:hi] (lo/hi multiples of 512)."""
            if hi <= TH:
                return st_sb[:, kt, lo:hi]
            assert lo >= TH
            return xT2_sb[:, kt, lo - TH:hi - TH]

        def wqv(kt, lo, hi):
            return st_sb[:, kt, TH + lo:TH + hi]

        def wkv(kt, lo, hi):
            return st_sb[:, kt, TH + CH + lo:TH + CH + hi]

        def wvv(kt):
            return st_sb[:, kt, TH + 2 * CH:TH + 3 * CH]

        # --- input DMAs: few dispatches, boot-ordered ----------------------
        for kt in range(2):
            nc.sync.dma_start(st_sb[:, kt], strip_r[kt])
        nc.sync.dma_start(msk_sb[:], msk)
        nc.sync.dma_start(bq_sb[:], bq_p)
        nc.sync.dma_start(bk_sb[:], bk_p)
        for kt in range(2, KT):
            nc.sync.dma_start(st_sb[:, kt], strip_r[kt])
        TQ = T // 4
        for kt in range(KT):  # tokens 1024:1536
            nc.sync.dma_start(xT2_sb[:, kt, 0:TQ], xT2_r[kt, :, 0:TQ])
        for kt in range(KT):  # tokens 1536:2048
            nc.sync.dma_start(xT2_sb[:, kt, TQ:2 * TQ], xT2_r[kt, :, TQ:2 * TQ])
        for ct in range(CT):
            nc.sync.dma_start(wo_sb[:, ct], wo_r[ct])
        nc.sync.dma_start(perm_sb[:], perm)

        # ones columns of V'
        for h in range(H_LOC):
            off = 64 if h % 2 == 0 else 0
            nc.gpsimd.memset(Vp_sb[:, :, h, off:off + 64], 1.0)

        # --- boot phase: kt-outer projection fills all 8 PSUM banks --------
        # Q(ct0, tq0-1), K(ct0, tq0-1): 2 banks each; V(tt0-3): 4 banks.
        with tc.tile_pool(name="boot", bufs=1, space="PSUM") as bp:
            qb_ps = bp.tile([P, 2 * TQB], fp32, name="qb_ps")
            kb_ps = bp.tile([P, 2 * TQB], fp32, name="kb_ps")
            vb_ps = bp.tile([P, 4, CH], fp32, name="vb_ps")
            for kt in range(KT):
                st, sp = (kt == 0), (kt == KT - 1)
                for tq in range(2):
                    ts = slice(tq * TQB, (tq + 1) * TQB)
                    nc.tensor.matmul(
                        qb_ps[:, ts], lhsT=wqv(kt, 0, P),
                        rhs=xv(kt, tq * TQB, (tq + 1) * TQB), start=st, stop=sp,
                        skip_group_check=True)
                    nc.tensor.matmul(
                        kb_ps[:, ts], lhsT=wkv(kt, 0, P),
                        rhs=xv(kt, tq * TQB, (tq + 1) * TQB), start=st, stop=sp,
                        skip_group_check=True)
                for tt in range(4):
                    nc.tensor.matmul(
                        vb_ps[:, tt], lhsT=xv(kt, tt * P, (tt + 1) * P),
                        rhs=wvv(kt), start=st, stop=sp,
                        skip_group_check=True)
            # finalize: Q/K biases on DVE, V copies split ACT/DVE (parallel)
            # so the PSUM banks recycle into the attention pools fast. V
            # carries no bias here: bv@Wo is folded into bo on the host.
            VpR = Vp_sb.rearrange("p tt (hp two) c -> p tt hp two c", two=2)
            v_r = vb_ps.rearrange("p tt (hp two c) -> p tt hp two c",
                                  two=2, c=64)
            nc.scalar.copy(out=VpR[:, 0:4, :, 0, 0:64], in_=v_r[:, :, :, 0, :])
            nc.scalar.copy(out=VpR[:, 0:4, :, 1, 64:128], in_=v_r[:, :, :, 1, :])
            for tq in range(2):
                ts = slice(tq * TQB, (tq + 1) * TQB)
                nc.vector.tensor_scalar_add(QT_sb[:, 0, ts], qb_ps[:, ts],
                                            bq_sb[:, 0:1])
            for tq in range(2):
                ts = slice(tq * TQB, (tq + 1) * TQB)
                nc.vector.tensor_scalar_add(KT_sb[:, 0, ts], kb_ps[:, ts],
                                            bk_sb[:, 0:1])

        with tc.tile_pool(name="psA", bufs=2, space="PSUM") as psA, \
             tc.tile_pool(name="psB", bufs=1, space="PSUM") as psB, \
             tc.tile_pool(name="wp", bufs=3) as wp:

            # ---- emission units (generators: yield every ~2 matmuls so the
            # drip scheduler can interleave filler at per-kb granularity) ---
            def emit_qk_unit(which, ct, tq):
                """One [128ch, 512tok] projection tile of Q^T or K^T."""
                lo, hi = tq * TQB, (tq + 1) * TQB
                wv_, b_sb, dst = ((wqv, bq_sb, QT_sb) if which == "q"
                                  else (wkv, bk_sb, KT_sb))
                p_ps = psA.tile([P, 2 * TQB], fp32, tag="s2", bufs=3, name="p_ps")[:, :TQB]
                for kt in range(KT):
                    nc.tensor.matmul(
                        p_ps, lhsT=wv_(kt, ct * P, (ct + 1) * P),
                        rhs=xv(kt, lo, hi),
                        start=(kt == 0), stop=(kt == KT - 1))
                    if kt % 2 == 1 and kt < KT - 1:
                        yield
                nc.vector.tensor_scalar_add(dst[:, ct, lo:hi], p_ps, b_sb[:, ct:ct + 1])

            def emit_v_unit(tt):
                """One [128tok, 512ch] V tile scattered into Vp (bias-free)."""
                v_ps = psA.tile([P, 2 * TQB], fp32, tag="s2", bufs=3, name="v_ps")[:, :CH]
                for kt in range(KT):
                    nc.tensor.matmul(
                        v_ps, lhsT=xv(kt, tt * P, (tt + 1) * P),
                        rhs=wvv(kt),
                        start=(kt == 0), stop=(kt == KT - 1))
                    if kt % 2 == 1 and kt < KT - 1:
                        yield
                v_r = v_ps.rearrange("p (hp two c) -> p hp two c", two=2, c=64)
                VpR2 = Vp_sb.rearrange("p tt (hp two) c -> p tt hp two c", two=2)
                nc.vector.tensor_copy(out=VpR2[:, tt, :, 0, 0:64], in_=v_r[:, :, 0, :])
                nc.vector.tensor_copy(out=VpR2[:, tt, :, 1, 64:128], in_=v_r[:, :, 1, :])

            def emit_outproj_unit(tt):
                """Both 512-halves of one [128tok, 1024] output row block.
                ct CT-1 accumulates last: its cxT may come from the chunk
                that just finished, whose scale-mults are still in flight."""
                o_ps = psA.tile([P, 2 * TQB], fp32, tag="s2", bufs=3, name="o_ps")
                for ct in range(CT):
                    for nh in range(2):
                        hs = slice(nh * TQB, (nh + 1) * TQB)
                        nc.tensor.matmul(
                            o_ps[:, hs], lhsT=cxT_sb[:, ct, tt * P:(tt + 1) * P],
                            rhs=wo_sb[:, ct, hs],
                            start=(ct == 0), stop=(ct == CT - 1),
                            skip_group_check=True)
                    if ct < CT - 1:
                        yield
                ob = wp.tile([P, 2 * TQB], bf16, tag="ob", bufs=3, name="ob")
                nc.vector.tensor_copy(out=ob, in_=o_ps)
                nc.sync.dma_start(outp_r[tt], ob)

            # filler queue: (stage, wave, genfn). stage = last chunk index
            # before which the unit must complete; wave = input wave the
            # unit's data arrives in (drip order is readiness-major).
            filler = []
            cur = [None]  # in-progress generator

            def _finish(gen):
                for _ in gen:
                    pass

            def drip(n=1):
                """Advance filler emission by n micro-steps (~2 matmuls)."""
                while n > 0:
                    if cur[0] is None:
                        if not filler:
                            return
                        _, _, mk = filler.pop(0)
                        cur[0] = mk()
                    try:
                        next(cur[0])
                        n -= 1
                    except StopIteration:
                        cur[0] = None

            def drip_boundary():
                """Chunk-boundary cover (~2us): finish the in-flight unit,
                then run one fresh unit. A second fresh allocation would just
                block on the s2 rotation until the next exp completes."""
                if cur[0] is not None:
                    _finish(cur[0])
                    cur[0] = None
                if filler:
                    _, _, mk = filler.pop(0)
                    _finish(mk())

            def flush(stage):
                if cur[0] is not None:
                    _finish(cur[0])
                    cur[0] = None
                i = 0
                while i < len(filler):
                    if filler[i][0] <= stage:
                        _, _, mk = filler.pop(i)
                        _finish(mk())
                    else:
                        i += 1

            # ---- attention ---------------------------------------------
            def emit_s_pair(pr, qb, kb):
                ks = slice(kb * P, (kb + 1) * P)
                v = kb - 4 * qb
                qoff = 128 * v if v > 0 else 0
                qsn = slice(qb * TQB + qoff, (qb + 1) * TQB)
                s2 = psA.tile([P, 2 * TQB], fp32, tag="s2", bufs=3, name="s2")
                nc.tensor.matmul(
                    s2[:, qoff:TQB],
                    lhsT=KT_sb[0:64, pr, ks], rhs=QT_sb[0:64, pr, qsn],
                    start=True, stop=True)
                nc.tensor.matmul(
                    s2[:, TQB + qoff:2 * TQB],
                    lhsT=KT_sb[64:128, pr, ks], rhs=QT_sb[64:128, pr, qsn],
                    start=True, stop=True)
                return s2

            def emit_exp(pr, qb, kb, s2):
                v = kb - 4 * qb
                qoff = 128 * v if v > 0 else 0
                s2r = s2.rearrange("p (two c) -> p two c", two=2)
                e2 = wp.tile([P, 2, TQB], bf16, tag="e2", bufs=8, name="e2")
                nc.scalar.activation(
                    e2[:, :, qoff:TQB], s2r[:, :, qoff:TQB], AF.Exp, scale=SCALE)
                if v >= 0:  # diagonal: triangular mask on first 128 q-cols
                    nc.gpsimd.tensor_tensor(
                        e2[:, :, qoff:qoff + P], e2[:, :, qoff:qoff + P],
                        msk_sb[:, None, :].to_broadcast((P, 2, P)), ALU.mult)
                return e2

            def emit_ctx(pr, qb, kb, nkb, e2, cx_e, cx_o):
                he, ho = 2 * pr, 2 * pr + 1
                v = kb - 4 * qb
                qoff = 128 * v if v > 0 else 0
                nc.tensor.matmul(
                    cx_e[:, qoff:TQB], lhsT=Vp_sb[:, kb, he, :],
                    rhs=e2[:, 0, qoff:TQB],
                    start=(kb == 0), stop=(kb == nkb - 1), skip_group_check=True)
                nc.tensor.matmul(
                    cx_o[:, qoff:TQB], lhsT=Vp_sb[:, kb, ho, :],
                    rhs=e2[:, 1, qoff:TQB],
                    start=(kb == 0), stop=(kb == nkb - 1), skip_group_check=True)

            deferred_mults = []
            pend_next = {}

            def emit_attention(pr, qb, nxt=None, pre_flush=None, fine_tail=False):
                qs = slice(qb * TQB, (qb + 1) * TQB)
                nkb = 4 * qb + 4
                cx = psB.tile([P, 2, TQB], fp32, tag="cx", name="cx")
                cx_e, cx_o = cx[:, 0], cx[:, 1]
                pend, e2_0 = pend_next.pop((pr, qb), (None, None))
                if pend is None:
                    pend = [emit_s_pair(pr, qb, kb) for kb in range(2)]
                # cover the previous normalize / first-exp latency bubble
                drip_boundary()
                for kb in range(nkb):
                    if kb + 2 < nkb:
                        pend.append(emit_s_pair(pr, qb, kb + 2))
                    e2 = e2_0 if (kb == 0 and e2_0 is not None) \
                        else emit_exp(pr, qb, kb, pend[kb])
                    emit_ctx(pr, qb, kb, nkb, e2, cx_e, cx_o)
                    if kb < nkb - 2 and kb % 2 == 0:
                        drip(1)
                # pre-emit the next chunk's first two score pairs + first exp
                # (ACT is exp-only, so the exp runs right behind this chunk's;
                # the next ctx0 then only waits on the csb copy below)
                if nxt is not None:
                    if pre_flush is not None:
                        pre_flush()
                    nsp = [emit_s_pair(nxt[0], nxt[1], kb) for kb in range(2)]
                    pend_next[nxt] = (nsp, emit_exp(nxt[0], nxt[1], 0, nsp[0]))
                # normalize: stage ctx/l out of PSUM in one wide DVE copy
                # (frees both cx banks; keeps ACT exp-only);
                # even head: ctx rows 0:64, l rows 64:128 / odd head mirrored.
                csb = wp.tile([P, 2, TQB], fp32, tag="csb", bufs=2, name="csb")
                nc.vector.tensor_copy(out=csb, in_=cx)
                cse, cso = csb[:, 0], csb[:, 1]
                lpair = wp.tile([P, TQB], fp32, tag="lpair", bufs=2, name="lpair")
                nc.vector.tensor_copy(out=lpair[64:128], in_=cse[64:128])
                nc.vector.tensor_copy(out=lpair[0:64], in_=cso[0:64])
                rec = wp.tile([P, TQB], fp32, tag="rec", bufs=2, name="rec")
                nc.vector.reciprocal_approx_fast(rec, lpair)
                if not fine_tail:
                    recs = wp.tile([P, TQB], fp32, tag="recs", bufs=2, name="recs")
                    nc.sync.dma_start(recs[0:64], rec[64:128])
                    nc.sync.dma_start(recs[64:128], rec[0:64])
                # previous qb's deferred scaling mults run AFTER this qb's
                # copies/recips so the DVE FIFO frees cx banks first
                while deferred_mults:
                    deferred_mults.pop(0)()

                if fine_tail:
                    # Last chunk (pr == CT-1): shorten the tail.
                    # - 1/l partition swap via a PE permutation matmul
                    #   instead of the slow SBUF-SBUF DMA
                    # - out-projection accumulates ct 0..2 early (their cxT
                    #   is old) and this chunk's ct last, per 128-row block
                    def op_early(tt):
                        o_ps = psA.tile([P, 2 * TQB], fp32, tag="s2",
                                        bufs=3, name="o_fin")
                        for nh in range(2):
                            hs = slice(nh * TQB, (nh + 1) * TQB)
                            for ct in range(CT - 1):
                                nc.tensor.matmul(
                                    o_ps[:, hs],
                                    lhsT=cxT_sb[:, ct, tt * P:(tt + 1) * P],
                                    rhs=wo_sb[:, ct, hs],
                                    start=(ct == 0), stop=False,
                                    skip_group_check=True)
                        return o_ps

                    def op_late(tt, o_ps):
                        for nh in range(2):
                            hs = slice(nh * TQB, (nh + 1) * TQB)
                            nc.tensor.matmul(
                                o_ps[:, hs],
                                lhsT=cxT_sb[:, CT - 1, tt * P:(tt + 1) * P],
                                rhs=wo_sb[:, CT - 1, hs],
                                start=False, stop=True,
                                skip_group_check=True)
                        ob = wp.tile([P, 2 * TQB], bf16, tag="ob", bufs=3,
                                     name="ob")
                        nc.vector.tensor_copy(out=ob, in_=o_ps)
                        nc.sync.dma_start(outp_r[tt], ob)

                    o01 = [op_early(4 * qb), op_early(4 * qb + 1)]
                    recs_ps = psB.tile([P, 2, TQB], fp32, tag="cx",
                                       name="recs_ps")[:, 0]
                    nc.tensor.matmul(recs_ps, lhsT=perm_sb, rhs=rec,
                                     start=True, stop=True)
                    for tf in range(4):
                        ts_ = slice(qb * TQB + tf * P, qb * TQB + (tf + 1) * P)
                        fs = slice(tf * P, (tf + 1) * P)
                        nc.vector.tensor_tensor(
                            cxT_sb[0:64, pr, ts_], cse[0:64, fs],
                            recs_ps[0:64, fs], ALU.mult)
                        nc.vector.tensor_tensor(
                            cxT_sb[64:128, pr, ts_], cso[64:128, fs],
                            recs_ps[64:128, fs], ALU.mult)
                        o_ps = o01[tf] if tf < 2 else op_early(4 * qb + tf)
                        op_late(4 * qb + tf, o_ps)
                    return

                def mults(pr=pr, qs=qs, cse=cse, cso=cso, recs=recs):
                    nc.vector.tensor_tensor(
                        cxT_sb[0:64, pr, qs], cse[0:64], recs[0:64], ALU.mult)
                    nc.vector.tensor_tensor(
                        cxT_sb[64:128, pr, qs], cso[64:128], recs[64:128], ALU.mult)
                deferred_mults.append(mults)

            # ---- schedule ----------------------------------------------
            # qb-major chunk order: all (pr, qb<=1) chunks first (these need
            # only boot + strip data), then qb=2, then qb=3. This keeps the
            # PE off the late xT waves early, and frees out-proj filler work
            # for the late phases.
            chunks = ([(pr, qb) for pr in range(CT) for qb in (0, 1)]
                      + [(pr, 2) for pr in range(CT)]
                      + [(pr, 3) for pr in range(CT)])
            pos = {c: i for i, c in enumerate(chunks)}

            # boot covered Q/K(ct0, tq0-1) and V(tt0-3).
            # wave: 0 = strip data (tok<1024), 1 = tok 1024:1536, 2 = rest.
            for tq in (2, 3):
                filler.append((pos[(0, tq)], tq - 1,
                               lambda tq=tq: emit_qk_unit("q", 0, tq)))
                filler.append((pos[(0, tq)], tq - 1,
                               lambda tq=tq: emit_qk_unit("k", 0, tq)))
            for tt in range(4, TT):
                wave = 0 if tt < 8 else (1 if tt < 12 else 2)
                filler.append((pos[(0, tt // 4)], wave,
                               lambda tt=tt: emit_v_unit(tt)))
            for ct in range(1, CT):
                for tq in range(NQ):
                    wave = 0 if tq < 2 else tq - 1
                    filler.append((pos[(ct, tq)], wave,
                                   lambda tq=tq, ct=ct: emit_qk_unit("q", ct, tq)))
                    filler.append((pos[(ct, tq)], wave,
                                   lambda tq=tq, ct=ct: emit_qk_unit("k", ct, tq)))
            filler.sort(key=lambda sf: (sf[1], sf[0]))

            for ci, (pr, qb) in enumerate(chunks):
                flush(ci)
                last = ci == len(chunks) - 1
                if last:
                    while filler:
                        drip()
                nxt = chunks[ci + 1] if not last else None
                emit_attention(pr, qb, nxt=nxt,
                               pre_flush=lambda ci=ci: flush(ci + 1),
                               fine_tail=last)
                if pr == CT - 1 and qb < NQ - 1:
                    # rows for this qb are now complete across all heads;
                    # emit this chunk's deferred cxT mults before any out-proj
                    # filler can be dripped against them
                    while deferred_mults:
                        deferred_mults.pop(0)()
                    for tt in range(4 * qb, 4 * qb + 4):
                        filler.append((99, 9, lambda tt=tt: emit_outproj_unit(tt)))

    nc.compile()
    return nc


def _get_nc():
    global _NC_CACHE
    if _NC_CACHE is None:
        _NC_CACHE = build_program()
    return _NC_CACHE


def _shard_inputs(input_Q, mask, Wq, bq, Wk, bk, Wv, bv, Wo):
    import ml_dtypes
    bf16 = ml_dtypes.bfloat16
    f32 = np.float32

    input_Q = np.asarray(input_Q, dtype=f32)
    mask = np.asarray(mask, dtype=bool)
    # causal-structure check: masks are derived assuming block-Toeplitz causal mask
    assert np.array_equal(mask, np.triu(np.ones((T, T), dtype=bool), k=1)), \
        "kernel assumes the standard causal mask"

    # triangular mask tile [128, 128]: keep iff p <= f  (k-offset p, q-offset f)
    keep = (~mask[0:P, 0:P]).astype(f32)              # [q, k]
    msk_np = np.ascontiguousarray(keep.T.astype(bf16))

    in_maps = []
    for c in range(N_CORES):
        b, g = c // HG, c % HG
        cs = slice(g * CH, (g + 1) * CH)
        xT_np = input_Q[b].T.astype(bf16)             # [D, T]
        wq_np = np.asarray(Wq, f32)[:, cs].astype(bf16)
        wk_np = np.asarray(Wk, f32)[:, cs].astype(bf16)
        wv_np = np.asarray(Wv, f32)[:, cs].astype(bf16)
        strip_np = np.ascontiguousarray(
            np.concatenate([xT_np[:, 0:TH], wq_np, wk_np, wv_np], axis=1))
        in_maps.append({
            "strip": strip_np,
            "xT2": np.ascontiguousarray(xT_np[:, TH:T]),
            "wo": np.ascontiguousarray(np.asarray(Wo, f32)[cs, :].astype(bf16)),
            "bq_p": np.ascontiguousarray(np.asarray(bq, f32)[cs].reshape(CT, P).T),
            "bk_p": np.ascontiguousarray(np.asarray(bk, f32)[cs].reshape(CT, P).T),
            "msk": msk_np,
            "perm": np.ascontiguousarray(
                np.roll(np.eye(P, dtype=f32), P // 2, axis=0)),
        })
    return in_maps


def _run(inputs, trace=False):
    from concourse.bass_utils import run_bass_kernel_spmd

    nc = _get_nc()
    in_maps = _shard_inputs(
        inputs["input_Q"], inputs["mask"], inputs["Wq"], inputs["bq"],
        inputs["Wk"], inputs["bk"], inputs["Wv"], inputs["bv"], inputs["Wo"])
    res = run_bass_kernel_spmd(
        nc, in_maps, core_ids=list(range(N_CORES)), trace=trace)

    # bv is folded here: concat_h(ctx_h/l_h + bv_h) @ Wo
    #                  = concat_h(ctx_h/l_h) @ Wo + bv @ Wo
    bo = (np.asarray(inputs["bo"], np.float32)
          + np.asarray(inputs["bv"], np.float32)
          @ np.asarray(inputs["Wo"], np.float32))
    out = np.empty((B, T, D_MODEL), np.float32)
    for b in range(B):
        out[b] = (np.asarray(res.results[2 * b]["outp"], np.float32)
                  + np.asarray(res.results[2 * b + 1]["outp"], np.float32)
                  + bo)
    return out, res


def kernel(**inputs):
    out, _ = _run(inputs, trace=False)
    return out
